# revision 10
# baseline (speedup 1.0000x reference)
import sys
import types

sys.path.insert(0, "/opt/trn_rl_repo")
import numpy as np
import ml_dtypes

BF = np.float16

N_NODES = 50000
N_EDGES = 600000
H = 128
EPSILON = 0.7071067811865476
EPS2 = EPSILON * EPSILON
EPS = 1e-08
NCORES = 8
NBLK = -(-(-(-N_NODES // NCORES)) // 128)   # node blocks per core
PERCORE = NBLK * 128                        # nodes per core (padded)
NPAD = NCORES * PERCORE                     # padded total nodes
NBTOT = NPAD // 128                         # total node blocks
GB = 8                                      # phase-A blocks per group
WJW = 520   # per-edge row: [A|B|C|onehot|dir0,dir1,dir2|pad]
NSPLIT = 32768                              # int16 gather split point
GCH = 8                                     # max tiles per dma_gather

SILU_NATIVE = True      # False: decompose silu into x*sigmoid(x) (for CoreSim)


def _wrap_idx(idxs):
    # [n] -> [128, n//16] int16: (p, c) = idxs[c*16 + p%16], replicated x8
    n = len(idxs)
    w = idxs.reshape(n // 16, 16).T
    return np.ascontiguousarray(np.tile(w, (8, 1)).astype(np.int16))


def _preprocess(inputs):
    s = np.asarray(inputs["s"], np.float32).reshape(N_NODES, H)
    v = np.asarray(inputs["v"], np.float32).reshape(N_NODES, 3 * H)
    dir_ij = np.asarray(inputs["dir_ij"], np.float32)
    Wij = np.asarray(inputs["Wij"], np.float32).reshape(N_EDGES, 3 * H)
    senders = np.asarray(inputs["senders"]).astype(np.int64)
    receivers = np.asarray(inputs["receivers"]).astype(np.int64)

    s_pad = np.zeros((NPAD, H), np.float32)
    s_pad[:N_NODES] = s
    v_pad = np.zeros((NPAD, 3 * H), np.float32)
    v_pad[:N_NODES] = v
    sT_bf = np.ascontiguousarray(s_pad.T).astype(BF)        # [128, NPAD]
    xv_tab = np.zeros((NPAD, 6 * H), BF)
    xv_tab[:, 3 * H:] = v_pad.astype(BF)

    owner = senders // PERCORE
    ls_all = senders - owner * PERCORE
    bb_all = ls_all // 128
    lp_all = ls_all % 128
    hi_all = (receivers >= NSPLIT).astype(np.int64)

    # group key: (block, hi-flag); count per core to size the tile groups
    cnt_lo = np.zeros((NCORES, NBLK), np.int64)
    cnt_hi = np.zeros((NCORES, NBLK), np.int64)
    for c in range(NCORES):
        m = owner == c
        cnt_lo[c] = np.bincount(bb_all[m & (hi_all == 0)], minlength=NBLK)
        cnt_hi[c] = np.bincount(bb_all[m & (hi_all == 1)], minlength=NBLK)
    tl = (-(-cnt_lo // 128)).max(axis=0)
    th = (-(-cnt_hi // 128)).max(axis=0)
    tiles_lo = [int(x) for x in tl]
    tiles_hi = [int(x) for x in th]
    tiles_b = [tiles_lo[b] + tiles_hi[b] for b in range(NBLK)]
    tile_base = np.concatenate([[0], np.cumsum(tiles_b)])
    t_total = int(tile_base[-1])
    rows_tot = t_total * 128

    shared = {
        "sT_bf": sT_bf,
        "xv_tab": xv_tab,
        "Wi1": np.asarray(inputs["Wi1"], np.float32).astype(BF),
        "bi1": np.asarray(inputs["bi1"], np.float32).reshape(H, 1),
        "Wi2": np.asarray(inputs["Wi2"], np.float32).astype(BF),
        "bi2": np.ascontiguousarray(np.broadcast_to(
            np.asarray(inputs["bi2"], np.float32).astype(BF).reshape(1, 3 * H),
            (128, 3 * H))),
        "Wm1a": np.ascontiguousarray(
            np.asarray(inputs["Wm1"], np.float32)[:H] * EPSILON).astype(BF),
        "Wm1b": np.ascontiguousarray(
            np.asarray(inputs["Wm1"], np.float32)[H:]).astype(BF),
        "bm1": np.asarray(inputs["bm1"], np.float32).reshape(H, 1),
        "Wm2": np.asarray(inputs["Wm2"], np.float32).astype(BF),
        "bm2": np.asarray(inputs["bm2"], np.float32).astype(BF).reshape(
            1, 3 * H),
        "Wvm": (np.asarray(inputs["Wvm"], np.float32) * EPSILON).astype(BF),
    }

    per_core = []
    for c in range(NCORES):
        sel = np.nonzero(owner == c)[0]
        # sort by (block, hi-flag, receiver)
        order = np.lexsort((receivers[sel], hi_all[sel], bb_all[sel]))
        sel = sel[order]
        bb = bb_all[sel]
        hi = hi_all[sel]
        key = bb * 2 + hi
        cnt = np.bincount(key, minlength=2 * NBLK)
        src = np.full(rows_tot, -1, np.int64)
        ofs = 0
        for b in range(NBLK):
            r0 = int(tile_base[b]) * 128
            n = int(cnt[2 * b])
            src[r0:r0 + n] = np.arange(ofs, ofs + n)
            ofs += n
            r1 = (int(tile_base[b]) + tiles_lo[b]) * 128
            n = int(cnt[2 * b + 1])
            src[r1:r1 + n] = np.arange(ofs, ofs + n)
            ofs += n
        mask = src >= 0
        rix = np.nonzero(mask)[0]
        gsel = sel[src[rix]]
        woh = np.zeros((rows_tot, WJW), BF)
        woh[rix, 0:384] = Wij[gsel].astype(BF)
        woh[rix, 384 + lp_all[gsel]] = 1.0
        woh[rix, 512:515] = dir_ij[gsel].astype(BF)
        idx16 = np.zeros(rows_tot, np.int64)
        rr = receivers[gsel]
        idx16[rix] = np.where(rr >= NSPLIT, rr - NSPLIT, rr)
        idxw = np.zeros((128, 8 * t_total), np.int16)
        for b in range(NBLK):
            t0 = int(tile_base[b])
            T = tiles_b[b]
            if T:
                idxw[:, 8 * t0:8 * (t0 + T)] = _wrap_idx(
                    idx16[t0 * 128:(t0 + T) * 128])
        sv_own = np.concatenate(
            [s_pad[c * PERCORE:(c + 1) * PERCORE],
             v_pad[c * PERCORE:(c + 1) * PERCORE]], axis=1)
        per_core.append({
            "woh": woh,
            "idxw": idxw,
            "sv_own": np.ascontiguousarray(sv_own).astype(BF),
        })
    bi2_zero = not np.any(np.asarray(inputs["bi2"]))
    return shared, per_core, tiles_lo, tiles_hi, t_total, bi2_zero


def _build(nc, tiles_lo, tiles_hi, t_total, bi2_zero=False):
    from concourse import bass, tile, mybir
    from concourse.masks import make_identity

    F32 = mybir.dt.float32
    BF16 = mybir.dt.float16
    I16 = mybir.dt.int16
    AF = mybir.ActivationFunctionType
    OP = mybir.AluOpType
    rows_tot = t_total * 128
    tiles_b = [tiles_lo[b] + tiles_hi[b] for b in range(NBLK)]
    TMAX = max(tiles_b) if tiles_b else 1
    tile_base = [0]
    for t in tiles_b:
        tile_base.append(tile_base[-1] + t)

    def dt(name, shape, dtype=F32, kind="ExternalInput"):
        return nc.dram_tensor(name, shape, dtype, kind=kind).ap()

    sT_d = dt("sT_bf", [H, NPAD], BF16)
    woh_d = dt("woh", [rows_tot, WJW], BF16)
    idxw_d = dt("idxw", [128, 8 * t_total], I16)
    sv_own_d = dt("sv_own", [PERCORE, 4 * H], BF16)
    wi1_d = dt("Wi1", [H, H], BF16)
    bi1_d = dt("bi1", [H, 1])
    wi2_d = dt("Wi2", [H, 3 * H], BF16)
    bi2_d = dt("bi2", [128, 3 * H], BF16)
    wm1a_d = dt("Wm1a", [H, H], BF16)
    wm1b_d = dt("Wm1b", [H, H], BF16)
    bm1_d = dt("bm1", [H, 1])
    wm2_d = dt("Wm2", [H, 3 * H], BF16)
    bm2_d = dt("bm2", [1, 3 * H], BF16)
    wvm_d = dt("Wvm", [H, 2 * H], BF16)
    xvtab_d = dt("xv_tab", [NPAD, 6 * H], BF16)
    out_d = dt("out", [PERCORE, 4 * H], kind="ExternalOutput")

    qrr = [0]

    def next_q():
        q = qrr[0]
        qrr[0] = (q + 1) % 4
        return q

    with tile.TileContext(nc) as tc:
        with tc.tile_pool(name="const", bufs=1) as cp:
            ident = cp.tile([128, 128], F32, name="ident")
            make_identity(nc, ident[:])
            ident16 = cp.tile([128, 128], BF16, name="ident16")
            nc.scalar.activation(out=ident16[:], in_=ident[:], func=AF.Copy)
            eps_t = cp.tile([128, 1], F32, name="eps_t")
            nc.vector.memset(eps_t[:], EPS)
            epsl_t = cp.tile([128, 1], F32, name="epsl_t")
            nc.vector.memset(epsl_t[:], EPSILON)
            ones1 = cp.tile([1, 128], F32, name="ones1")
            nc.vector.memset(ones1[:], 1.0)
            ones1b = cp.tile([1, 128], BF16, name="ones1b")
            nc.vector.memset(ones1b[:], 1.0)
            idxw_t = cp.tile([128, 8 * t_total], I16, name="idxw_t")
            nc.scalar.dma_start(out=idxw_t[:], in_=idxw_d[:])

            def load(name, dram, shape, dtype=F32):
                t = cp.tile(shape, dtype, name=name)
                nc.sync.dma_start(out=t[:], in_=dram[:])
                return t

            wi1_t = load("wi1_t", wi1_d, [H, H], BF16)
            bi1_t = load("bi1_t", bi1_d, [H, 1])
            wi2_t = load("wi2_t", wi2_d, [H, 3 * H], BF16)
            bi2_t = load("bi2_t", bi2_d, [128, 3 * H], BF16)
            wm1a_t = load("wm1a_t", wm1a_d, [H, H], BF16)
            wm1b_t = load("wm1b_t", wm1b_d, [H, H], BF16)
            bm1_t = load("bm1_t", bm1_d, [H, 1])
            wm2_t = load("wm2_t", wm2_d, [H, 3 * H], BF16)
            bm2_t = load("bm2_t", bm2_d, [1, 3 * H], BF16)
            wvm_t = load("wvm_t", wvm_d, [H, 2 * H], BF16)

            def silu(pool, out_ap, in_ps_ap, bias_ap, shape, tag):
                if SILU_NATIVE:
                    nc.scalar.activation(out=out_ap, in_=in_ps_ap, func=AF.Silu,
                                         bias=bias_ap)
                else:
                    z = pool.tile(shape, F32, name=tag + "_z")
                    nc.vector.tensor_scalar(out=z[:], in0=in_ps_ap,
                                            scalar1=bias_ap, scalar2=None,
                                            op0=OP.add)
                    sg = pool.tile(shape, F32, name=tag + "_sg")
                    nc.scalar.activation(out=sg[:], in_=in_ps_ap,
                                         func=AF.Sigmoid, bias=bias_ap)
                    nc.vector.tensor_tensor(out=out_ap, in0=z[:], in1=sg[:],
                                            op=OP.mult)

            # ---- Phase A: x_tab[n] = MLP_i(s)[n]  (transpose-free) --------
            with tc.tile_pool(name="pa", bufs=2) as pa, \
                 tc.tile_pool(name="psa", bufs=2, space="PSUM") as psa:
                for g in range(NBTOT // GB):
                    r0 = g * GB * 128
                    sT_g = pa.tile([128, GB * 128], BF16, name="sT_g")
                    nc.sync.dma_start(out=sT_g[:],
                                      in_=sT_d[:, r0:r0 + GB * 128])
                    x8 = pa.tile([128, GB, 3 * H], BF16, name="x8")
                    for half in range(GB // 4):
                        hps4 = psa.tile([128, 512], F32, name="hps4")
                        nc.tensor.matmul(hps4[:], lhsT=wi1_t[:],
                                         rhs=sT_g[:, half * 512:(half + 1) * 512],
                                         start=True, stop=True)
                        hs4 = pa.tile([128, 512], BF16, name="hs4", bufs=4)
                        silu(pa, hs4[:], hps4[:], bi1_t[:], [128, 512], "sa")
                        for jj in range(4):
                            j = half * 4 + jj
                            xps = psa.tile([128, 3 * H], F32, name="xps",
                                           bufs=4)
                            nc.tensor.matmul(
                                xps[:],
                                lhsT=hs4[:, jj * 128:(jj + 1) * 128],
                                rhs=wi2_t[:],
                                start=True, stop=True)
                            if bi2_zero:
                                nc.scalar.activation(out=x8[:, j, :],
                                                     in_=xps[:], func=AF.Copy)
                            else:
                                nc.vector.tensor_tensor(out=x8[:, j, :],
                                                        in0=xps[:],
                                                        in1=bi2_t[:],
                                                        op=OP.add)
                    nc.sync.dma_start(
                        out=xvtab_d[r0:r0 + GB * 128, 0:3 * H].rearrange(
                            "(j p) f -> p j f", p=128),
                        in_=x8[:])
            # ---- Phase B + C: messages, scatter, update -------------------
            with tc.tile_pool(name="pb", bufs=2) as pb, \
                 tc.tile_pool(name="pc", bufs=2) as pcp, \
                 tc.tile_pool(name="psb", bufs=2, space="PSUM") as psb, \
                 tc.tile_pool(name="psc", bufs=1, space="PSUM") as psc:
                for b in range(NBLK):
                    T = tiles_b[b]
                    TL = tiles_lo[b]
                    THI = tiles_hi[b]
                    t0 = tile_base[b]
                    pblk = psb.tile([128, 512], F32, name="pblk")
                    if T > 0:
                        w_t = pb.tile([128, TMAX, WJW], BF16, name="w_t")
                        nc.sync.dma_start(
                            out=w_t[:, 0:T, :],
                            in_=woh_d[t0 * 128:(t0 + T) * 128, :]
                            .rearrange("(j p) f -> p j f", p=128))
                        xvg = pb.tile([128, TMAX, 6 * H], BF16,
                                      name="xvg")
                        for c0 in range(0, TL, GCH):
                            cs = min(GCH, TL - c0)
                            nc.gpsimd.dma_gather(
                                out_ap=xvg[:, c0:c0 + cs, :],
                                in_ap=xvtab_d[:],
                                idxs_ap=idxw_t[:, 8 * (t0 + c0):
                                               8 * (t0 + c0 + cs)],
                                num_idxs=cs * 128,
                                num_idxs_reg=cs * 128,
                                elem_size=6 * H, queue_num=next_q())
                        for c0 in range(0, THI, GCH):
                            cs = min(GCH, THI - c0)
                            a0 = TL + c0
                            nc.gpsimd.dma_gather(
                                out_ap=xvg[:, a0:a0 + cs, :],
                                in_ap=xvtab_d[NSPLIT:NPAD, :],
                                idxs_ap=idxw_t[:, 8 * (t0 + a0):
                                               8 * (t0 + a0 + cs)],
                                num_idxs=cs * 128,
                                num_idxs_reg=cs * 128,
                                elem_size=6 * H, queue_num=next_q())
                        # per-edge message math (all bf16)
                        qm = pb.tile([128, TMAX, 896], BF16, name="qm")
                        t1d = pb.tile([128, TMAX, 3 * H], BF16, name="t1d")
                        # [ds|t1|t2] = [A|B|C] * [x0|x1|x2] in one op
                        nc.vector.tensor_tensor(
                            out=qm[:, 0:T, 384:768], in0=w_t[:, 0:T, 0:384],
                            in1=xvg[:, 0:T, 0:384], op=OP.mult)
                        # dv2 * vj  (t2 broadcast over the 3 dirs)
                        t2b = qm[:, 0:T, 640:768].unsqueeze(2).broadcast_to(
                            [128, T, 3, 128])
                        nc.vector.tensor_tensor(
                            out=qm[:, 0:T, 0:384].rearrange(
                                "p j (d f) -> p j d f", d=3),
                            in0=t2b, in1=xvg[:, 0:T, 3 * H:6 * H]
                            .rearrange("p j (d f) -> p j d f", d=3),
                            op=OP.mult)
                        # dv1 * dir_d
                        t1b = qm[:, 0:T, 512:640].unsqueeze(2).broadcast_to(
                            [128, T, 3, 128])
                        dirb = w_t[:, 0:T, 512:515].unsqueeze(3).broadcast_to(
                            [128, T, 3, 128])
                        nc.vector.tensor_tensor(
                            out=t1d[:, 0:T, :].rearrange(
                                "p j (d f) -> p j d f", d=3),
                            in0=t1b, in1=dirb, op=OP.mult)
                        nc.vector.tensor_tensor(
                            out=qm[:, 0:T, 0:384], in0=qm[:, 0:T, 0:384],
                            in1=t1d[:, 0:T, :], op=OP.add)
                        # scatter-accumulate into the block PSUM
                        for j in range(T):
                            nc.tensor.matmul(pblk[:],
                                             lhsT=w_t[:, j, 384:512],
                                             rhs=qm[:, j, 0:512],
                                             start=(j == 0),
                                             stop=(j == T - 1))
                    # ---- Phase C for block b ----
                    sv_t = pcp.tile([128, 4 * H], BF16, name="sv_t")
                    nc.scalar.dma_start(out=sv_t[:],
                                        in_=sv_own_d[b * 128:(b + 1) * 128, :])
                    sv_raw = pcp.tile([128, H], BF16, name="sv_raw")
                    vsum = pcp.tile([128, 3 * H], BF16, name="vsum")
                    if T > 0:
                        nc.vector.tensor_tensor(out=sv_raw[:],
                                                in0=sv_t[:, 0:128],
                                                in1=pblk[:, 384:512],
                                                op=OP.add)
                        nc.vector.tensor_tensor(out=vsum[:],
                                                in0=sv_t[:, 128:512],
                                                in1=pblk[:, 0:384],
                                                op=OP.add)
                    else:
                        nc.vector.tensor_copy(out=sv_raw[:], in_=sv_t[:, 0:128])
                        nc.vector.tensor_copy(out=vsum[:], in_=sv_t[:, 128:512])
                    vwd = []
                    sq = []
                    for d in range(3):
                        trc = psc.tile([128, 128], BF16, name="trc",
                                       bufs=2)
                        nc.tensor.transpose(
                            trc[:],
                            in_=vsum[:, d * 128:(d + 1) * 128],
                            identity=ident16[:])
                        vT = pcp.tile([128, 128], BF16, name="vT", bufs=6)
                        nc.scalar.activation(out=vT[:], in_=trc[:],
                                             func=AF.Copy)
                        vw = psc.tile([128, 2 * H], F32, name="vw", bufs=2)
                        nc.tensor.matmul(vw[:], lhsT=vT[:],
                                         rhs=wvm_t[:],
                                         start=True, stop=True)
                        vws = pcp.tile([128, 2 * H], BF16, name="vws",
                                       bufs=6)
                        nc.scalar.activation(out=vws[:], in_=vw[:],
                                             func=AF.Copy)
                        vwd.append(vws)
                        sq_d = pcp.tile([128, 128], F32, name="sq", bufs=6)
                        nc.vector.tensor_tensor(out=sq_d[:],
                                                in0=vws[:, 128:256],
                                                in1=vws[:, 128:256],
                                                op=OP.mult)
                        sq.append(sq_d)
                    acc = pcp.tile([128, 128], F32, name="acc")
                    nc.vector.tensor_tensor(out=acc[:], in0=sq[0][:],
                                            in1=sq[1][:], op=OP.add)
                    nc.vector.tensor_tensor(out=acc[:], in0=acc[:],
                                            in1=sq[2][:], op=OP.add)
                    vnorm = pcp.tile([128, 128], BF16, name="vnorm")
                    nc.scalar.activation(out=vnorm[:], in_=acc[:],
                                         func=AF.Sqrt, bias=eps_t[:])
                    hps = psc.tile([128, 128], F32, name="hps")
                    for k, src_t in enumerate((sv_raw, vnorm)):
                        trc = psc.tile([128, 128], BF16, name="trc", bufs=2)
                        nc.tensor.transpose(trc[:],
                                            in_=src_t[:],
                                            identity=ident16[:])
                        tsT = pcp.tile([128, 128], BF16, name="tsT", bufs=4)
                        nc.scalar.activation(out=tsT[:], in_=trc[:],
                                             func=AF.Copy)
                        lhs = wm1a_t if k == 0 else wm1b_t
                        nc.tensor.matmul(hps[:], lhsT=lhs[:],
                                         rhs=tsT[:],
                                         start=(k == 0), stop=(k == 1))
                    hsb = pcp.tile([128, 128], BF16, name="hsb")
                    silu(pcp, hsb[:], hps[:], bm1_t[:], [128, 128], "sc")
                    ops_ = psc.tile([128, 3 * H], F32, name="ops")
                    nc.tensor.matmul(ops_[:], lhsT=ones1b[:],
                                     rhs=bm2_t[:],
                                     start=True, stop=False)
                    nc.tensor.matmul(ops_[:], lhsT=hsb[:],
                                     rhs=wm2_t[:],
                                     start=False, stop=True)
                    svl = pcp.tile([128, 128], F32, name="svl")
                    m2 = pcp.tile([128, 128], F32, name="m2")
                    nc.vector.tensor_tensor(out=svl[:], in0=vwd[0][:, 0:128],
                                            in1=vwd[0][:, 128:256],
                                            op=OP.mult)
                    nc.vector.tensor_tensor(out=m2[:], in0=vwd[1][:, 0:128],
                                            in1=vwd[1][:, 128:256],
                                            op=OP.mult)
                    nc.vector.tensor_tensor(out=svl[:], in0=svl[:], in1=m2[:],
                                            op=OP.add)
                    nc.vector.tensor_tensor(out=m2[:], in0=vwd[2][:, 0:128],
                                            in1=vwd[2][:, 128:256],
                                            op=OP.mult)
                    nc.vector.tensor_tensor(out=svl[:], in0=svl[:], in1=m2[:],
                                            op=OP.add)
                    dsv = pcp.tile([128, 128], F32, name="dsv")
                    nc.vector.tensor_tensor(out=dsv[:], in0=ops_[:, 256:384],
                                            in1=svl[:], op=OP.mult)
                    accs = pcp.tile([128, 128], F32, name="accs")
                    nc.vector.tensor_tensor(out=accs[:], in0=ops_[:, 0:128],
                                            in1=dsv[:], op=OP.add)
                    outt = pcp.tile([128, 4 * H], F32, name="outt")
                    t1 = pcp.tile([128, 128], F32, name="t1")
                    nc.scalar.activation(out=t1[:], in_=accs[:], func=AF.Copy,
                                         scale=epsl_t[:])
                    t2 = pcp.tile([128, 128], F32, name="t2")
                    nc.scalar.activation(out=t2[:], in_=sv_raw[:],
                                         func=AF.Copy, scale=EPS2)
                    nc.vector.tensor_tensor(out=outt[:, 0:128], in0=t1[:],
                                            in1=t2[:], op=OP.add)
                    for d in range(3):
                        q = pcp.tile([128, 128], F32, name="qd", bufs=6)
                        nc.vector.tensor_tensor(out=q[:],
                                                in0=vwd[d][:, 0:128],
                                                in1=ops_[:, 128:256],
                                                op=OP.mult)
                        qs = pcp.tile([128, 128], F32, name="qsd", bufs=6)
                        nc.scalar.activation(out=qs[:], in_=q[:], func=AF.Copy,
                                             scale=epsl_t[:])
                        r = pcp.tile([128, 128], F32, name="rd", bufs=6)
                        nc.scalar.activation(
                            out=r[:], in_=vsum[:, d * 128:(d + 1) * 128],
                            func=AF.Copy, scale=EPS2)
                        nc.vector.tensor_tensor(
                            out=outt[:, 128 + d * 128:256 + d * 128],
                            in0=qs[:], in1=r[:], op=OP.add)
                    nc.sync.dma_start(out=out_d[b * 128:(b + 1) * 128, :],
                                      in_=outt[:])


def _install_trace_hook():
    try:
        import antenv
        if "antenv.axon_hooks" not in sys.modules:
            mod = types.ModuleType("antenv.axon_hooks")
            mod._hook = None

            def set_axon_ntff_profile_hook(h):
                mod._hook = h

            def get_axon_ntff_profile_hook():
                return mod._hook

            mod.set_axon_ntff_profile_hook = set_axon_ntff_profile_hook
            mod.get_axon_ntff_profile_hook = get_axon_ntff_profile_hook
            sys.modules["antenv.axon_hooks"] = mod
            antenv.axon_hooks = mod
        from antenv.axon_hooks import (get_axon_ntff_profile_hook,
                                       set_axon_ntff_profile_hook)
        if get_axon_ntff_profile_hook() is None:
            from trn_agent_boot.trn_boot import _ntff_profile_via_ctypes
            set_axon_ntff_profile_hook(
                _ntff_profile_via_ctypes("/opt/axon/libaxon_pjrt.so"))
        return True
    except Exception:
        return False


def kernel(**inputs):
    from concourse import bacc
    from concourse.bass_utils import run_bass_kernel_spmd

    (shared, per_core, tiles_lo, tiles_hi, t_total,
     bi2_zero) = _preprocess(inputs)
    nc = bacc.Bacc("TRN2", target_bir_lowering=False, debug=False,
                   num_devices=NCORES, num_swdge_queues=4)
    _build(nc, tiles_lo, tiles_hi, t_total, bi2_zero)
    nc.compile()

    in_maps = [dict(shared, **per_core[c]) for c in range(NCORES)]
    trace = _install_trace_hook()
    try:
        res = run_bass_kernel_spmd(nc, in_maps, core_ids=list(range(NCORES)),
                                   trace=trace)
    except Exception:
        if not trace:
            raise
        res = run_bass_kernel_spmd(nc, in_maps, core_ids=list(range(NCORES)),
                                   trace=False)
    kernel.last_exec_time_ns = getattr(res, "exec_time_ns", None)
    outs = [np.asarray(res.results[c]["out"]) for c in range(NCORES)]
    full = np.concatenate(outs, axis=0)[:N_NODES]
    return np.ascontiguousarray(full.reshape(N_NODES, 4, H), dtype=np.float32)


# revision 11
# speedup vs baseline: 1.0970x; 1.0970x over previous
import sys
import types

sys.path.insert(0, "/opt/trn_rl_repo")
import numpy as np
import ml_dtypes

BF = np.float16

N_NODES = 50000
N_EDGES = 600000
H = 128
EPSILON = 0.7071067811865476
EPS2 = EPSILON * EPSILON
EPS = 1e-08
NCORES = 8
NBLK = -(-(-(-N_NODES // NCORES)) // 128)   # node blocks per core
PERCORE = NBLK * 128                        # nodes per core (padded)
NPAD = NCORES * PERCORE                     # padded total nodes
NBTOT = NPAD // 128                         # total node blocks
GB = 8                                      # phase-A blocks per group
WJW = 520   # per-edge row: [A|B|C|onehot|dir0,dir1,dir2|pad]
NSPLIT = 32768                              # int16 gather split point
GCH = 8                                     # max tiles per dma_gather

SILU_NATIVE = True      # False: decompose silu into x*sigmoid(x) (for CoreSim)


def _wrap_idx(idxs):
    # [n] -> [128, n//16] int16: (p, c) = idxs[c*16 + p%16], replicated x8
    n = len(idxs)
    w = idxs.reshape(n // 16, 16).T
    return np.ascontiguousarray(np.tile(w, (8, 1)).astype(np.int16))


def _preprocess(inputs):
    s = np.asarray(inputs["s"], np.float32).reshape(N_NODES, H)
    v = np.asarray(inputs["v"], np.float32).reshape(N_NODES, 3 * H)
    dir_ij = np.asarray(inputs["dir_ij"], np.float32)
    Wij = np.asarray(inputs["Wij"], np.float32).reshape(N_EDGES, 3 * H)
    senders = np.asarray(inputs["senders"]).astype(np.int64)
    receivers = np.asarray(inputs["receivers"]).astype(np.int64)

    s_pad = np.zeros((NPAD, H), np.float32)
    s_pad[:N_NODES] = s
    v_pad = np.zeros((NPAD, 3 * H), np.float32)
    v_pad[:N_NODES] = v
    sT_bf = np.ascontiguousarray(s_pad.T).astype(BF)        # [128, NPAD]
    xv_tab = np.zeros((NPAD, 6 * H), BF)
    xv_tab[:, 3 * H:] = v_pad.astype(BF)

    owner = senders // PERCORE
    ls_all = senders - owner * PERCORE
    bb_all = ls_all // 128
    lp_all = ls_all % 128
    hi_all = (receivers >= NSPLIT).astype(np.int64)

    # group key: (block, hi-flag); count per core to size the tile groups
    cnt_lo = np.zeros((NCORES, NBLK), np.int64)
    cnt_hi = np.zeros((NCORES, NBLK), np.int64)
    for c in range(NCORES):
        m = owner == c
        cnt_lo[c] = np.bincount(bb_all[m & (hi_all == 0)], minlength=NBLK)
        cnt_hi[c] = np.bincount(bb_all[m & (hi_all == 1)], minlength=NBLK)
    tl = (-(-cnt_lo // 128)).max(axis=0)
    th = (-(-cnt_hi // 128)).max(axis=0)
    tiles_lo = [int(x) for x in tl]
    tiles_hi = [int(x) for x in th]
    tiles_b = [tiles_lo[b] + tiles_hi[b] for b in range(NBLK)]
    tile_base = np.concatenate([[0], np.cumsum(tiles_b)])
    t_total = int(tile_base[-1])
    rows_tot = t_total * 128

    shared = {
        "sT_bf": sT_bf,
        "xv_tab": xv_tab,
        "Wi1": np.asarray(inputs["Wi1"], np.float32).astype(BF),
        "bi1": np.asarray(inputs["bi1"], np.float32).reshape(H, 1),
        "Wi2": np.asarray(inputs["Wi2"], np.float32).astype(BF),
        "bi2": np.ascontiguousarray(np.broadcast_to(
            np.asarray(inputs["bi2"], np.float32).astype(BF).reshape(1, 3 * H),
            (128, 3 * H))),
        "Wm1a": np.ascontiguousarray(
            np.asarray(inputs["Wm1"], np.float32)[:H] * EPSILON).astype(BF),
        "Wm1b": np.ascontiguousarray(
            np.asarray(inputs["Wm1"], np.float32)[H:]).astype(BF),
        "bm1": np.asarray(inputs["bm1"], np.float32).reshape(H, 1),
        "Wm2": np.asarray(inputs["Wm2"], np.float32).astype(BF),
        "bm2": np.asarray(inputs["bm2"], np.float32).astype(BF).reshape(
            1, 3 * H),
        "Wvm": (np.asarray(inputs["Wvm"], np.float32) * EPSILON).astype(BF),
    }

    per_core = []
    for c in range(NCORES):
        sel = np.nonzero(owner == c)[0]
        # sort by (block, hi-flag, receiver)
        order = np.lexsort((receivers[sel], hi_all[sel], bb_all[sel]))
        sel = sel[order]
        bb = bb_all[sel]
        hi = hi_all[sel]
        key = bb * 2 + hi
        cnt = np.bincount(key, minlength=2 * NBLK)
        src = np.full(rows_tot, -1, np.int64)
        ofs = 0
        for b in range(NBLK):
            r0 = int(tile_base[b]) * 128
            n = int(cnt[2 * b])
            src[r0:r0 + n] = np.arange(ofs, ofs + n)
            ofs += n
            r1 = (int(tile_base[b]) + tiles_lo[b]) * 128
            n = int(cnt[2 * b + 1])
            src[r1:r1 + n] = np.arange(ofs, ofs + n)
            ofs += n
        mask = src >= 0
        rix = np.nonzero(mask)[0]
        gsel = sel[src[rix]]
        woh = np.zeros((rows_tot, WJW), BF)
        woh[rix, 0:384] = Wij[gsel].astype(BF)
        woh[rix, 384 + lp_all[gsel]] = 1.0
        woh[rix, 512:515] = dir_ij[gsel].astype(BF)
        idx16 = np.zeros(rows_tot, np.int64)
        rr = receivers[gsel]
        idx16[rix] = np.where(rr >= NSPLIT, rr - NSPLIT, rr)
        idxw = np.zeros((128, 8 * t_total), np.int16)
        for b in range(NBLK):
            t0 = int(tile_base[b])
            T = tiles_b[b]
            if T:
                idxw[:, 8 * t0:8 * (t0 + T)] = _wrap_idx(
                    idx16[t0 * 128:(t0 + T) * 128])
        sv_own = np.concatenate(
            [v_pad[c * PERCORE:(c + 1) * PERCORE],
             s_pad[c * PERCORE:(c + 1) * PERCORE]], axis=1)
        per_core.append({
            "woh": woh,
            "idxw": idxw,
            "sv_own": np.ascontiguousarray(sv_own).astype(BF),
        })
    bi2_zero = not np.any(np.asarray(inputs["bi2"]))
    return shared, per_core, tiles_lo, tiles_hi, t_total, bi2_zero


def _build(nc, tiles_lo, tiles_hi, t_total, bi2_zero=False):
    from concourse import bass, tile, mybir
    from concourse.masks import make_identity

    F32 = mybir.dt.float32
    BF16 = mybir.dt.float16
    I16 = mybir.dt.int16
    AF = mybir.ActivationFunctionType
    OP = mybir.AluOpType
    rows_tot = t_total * 128
    tiles_b = [tiles_lo[b] + tiles_hi[b] for b in range(NBLK)]
    TMAX = max(tiles_b) if tiles_b else 1
    tile_base = [0]
    for t in tiles_b:
        tile_base.append(tile_base[-1] + t)

    def dt(name, shape, dtype=F32, kind="ExternalInput"):
        return nc.dram_tensor(name, shape, dtype, kind=kind).ap()

    sT_d = dt("sT_bf", [H, NPAD], BF16)
    woh_d = dt("woh", [rows_tot, WJW], BF16)
    idxw_d = dt("idxw", [128, 8 * t_total], I16)
    sv_own_d = dt("sv_own", [PERCORE, 4 * H], BF16)
    wi1_d = dt("Wi1", [H, H], BF16)
    bi1_d = dt("bi1", [H, 1])
    wi2_d = dt("Wi2", [H, 3 * H], BF16)
    bi2_d = dt("bi2", [128, 3 * H], BF16)
    wm1a_d = dt("Wm1a", [H, H], BF16)
    wm1b_d = dt("Wm1b", [H, H], BF16)
    bm1_d = dt("bm1", [H, 1])
    wm2_d = dt("Wm2", [H, 3 * H], BF16)
    bm2_d = dt("bm2", [1, 3 * H], BF16)
    wvm_d = dt("Wvm", [H, 2 * H], BF16)
    xvtab_d = dt("xv_tab", [NPAD, 6 * H], BF16)
    out_d = dt("out", [PERCORE, 4 * H], BF16,
               kind="ExternalOutput")

    qrr = [0]

    def next_q():
        q = qrr[0]
        qrr[0] = (q + 1) % 4
        return q

    with tile.TileContext(nc) as tc:
        with tc.tile_pool(name="const", bufs=1) as cp:
            ident = cp.tile([128, 128], F32, name="ident")
            make_identity(nc, ident[:])
            ident16 = cp.tile([128, 128], BF16, name="ident16")
            nc.scalar.activation(out=ident16[:], in_=ident[:], func=AF.Copy)
            eps_t = cp.tile([128, 1], F32, name="eps_t")
            nc.vector.memset(eps_t[:], EPS)
            epsl_t = cp.tile([128, 1], F32, name="epsl_t")
            nc.vector.memset(epsl_t[:], EPSILON)
            ones1 = cp.tile([1, 128], F32, name="ones1")
            nc.vector.memset(ones1[:], 1.0)
            ones1b = cp.tile([1, 128], BF16, name="ones1b")
            nc.vector.memset(ones1b[:], 1.0)
            idxw_t = cp.tile([128, 8 * t_total], I16, name="idxw_t")
            nc.scalar.dma_start(out=idxw_t[:], in_=idxw_d[:])

            def load(name, dram, shape, dtype=F32):
                t = cp.tile(shape, dtype, name=name)
                nc.sync.dma_start(out=t[:], in_=dram[:])
                return t

            wi1_t = load("wi1_t", wi1_d, [H, H], BF16)
            bi1_t = load("bi1_t", bi1_d, [H, 1])
            wi2_t = load("wi2_t", wi2_d, [H, 3 * H], BF16)
            bi2_t = load("bi2_t", bi2_d, [128, 3 * H], BF16)
            wm1a_t = load("wm1a_t", wm1a_d, [H, H], BF16)
            wm1b_t = load("wm1b_t", wm1b_d, [H, H], BF16)
            bm1_t = load("bm1_t", bm1_d, [H, 1])
            wm2_t = load("wm2_t", wm2_d, [H, 3 * H], BF16)
            bm2_t = load("bm2_t", bm2_d, [1, 3 * H], BF16)
            wvm_t = load("wvm_t", wvm_d, [H, 2 * H], BF16)

            def silu(pool, out_ap, in_ps_ap, bias_ap, shape, tag):
                if SILU_NATIVE:
                    nc.scalar.activation(out=out_ap, in_=in_ps_ap, func=AF.Silu,
                                         bias=bias_ap)
                else:
                    z = pool.tile(shape, F32, name=tag + "_z")
                    nc.vector.tensor_scalar(out=z[:], in0=in_ps_ap,
                                            scalar1=bias_ap, scalar2=None,
                                            op0=OP.add)
                    sg = pool.tile(shape, F32, name=tag + "_sg")
                    nc.scalar.activation(out=sg[:], in_=in_ps_ap,
                                         func=AF.Sigmoid, bias=bias_ap)
                    nc.vector.tensor_tensor(out=out_ap, in0=z[:], in1=sg[:],
                                            op=OP.mult)

            # ---- Phase A: x_tab[n] = MLP_i(s)[n]  (transpose-free) --------
            with tc.tile_pool(name="pa", bufs=2) as pa, \
                 tc.tile_pool(name="psa", bufs=2, space="PSUM") as psa:
                for g in range(NBTOT // GB):
                    r0 = g * GB * 128
                    sT_g = pa.tile([128, GB * 128], BF16, name="sT_g")
                    nc.sync.dma_start(out=sT_g[:],
                                      in_=sT_d[:, r0:r0 + GB * 128])
                    x8 = pa.tile([128, GB, 3 * H], BF16, name="x8")
                    for half in range(GB // 4):
                        hps4 = psa.tile([128, 512], F32, name="hps4")
                        nc.tensor.matmul(hps4[:], lhsT=wi1_t[:],
                                         rhs=sT_g[:, half * 512:(half + 1) * 512],
                                         start=True, stop=True)
                        hs4 = pa.tile([128, 512], BF16, name="hs4", bufs=4)
                        silu(pa, hs4[:], hps4[:], bi1_t[:], [128, 512], "sa")
                        for jj in range(4):
                            j = half * 4 + jj
                            xps = psa.tile([128, 3 * H], F32, name="xps",
                                           bufs=4)
                            nc.tensor.matmul(
                                xps[:],
                                lhsT=hs4[:, jj * 128:(jj + 1) * 128],
                                rhs=wi2_t[:],
                                start=True, stop=True)
                            if bi2_zero:
                                nc.scalar.activation(out=x8[:, j, :],
                                                     in_=xps[:], func=AF.Copy)
                            else:
                                nc.vector.tensor_tensor(out=x8[:, j, :],
                                                        in0=xps[:],
                                                        in1=bi2_t[:],
                                                        op=OP.add)
                    nc.sync.dma_start(
                        out=xvtab_d[r0:r0 + GB * 128, 0:3 * H].rearrange(
                            "(j p) f -> p j f", p=128),
                        in_=x8[:])
            # ---- Phase B + C: messages, scatter, update -------------------
            with tc.tile_pool(name="pb", bufs=2) as pb, \
                 tc.tile_pool(name="pc", bufs=2) as pcp, \
                 tc.tile_pool(name="psb", bufs=2, space="PSUM") as psb, \
                 tc.tile_pool(name="psc", bufs=1, space="PSUM") as psc:
                for b in range(NBLK):
                    T = tiles_b[b]
                    TL = tiles_lo[b]
                    THI = tiles_hi[b]
                    t0 = tile_base[b]
                    pblk = psb.tile([128, 512], F32, name="pblk")
                    svp_t = pcp.tile([128, 4 * H], BF16, name="svp_t")
                    nc.scalar.dma_start(out=svp_t[:],
                                        in_=sv_own_d[b * 128:(b + 1) * 128, :])
                    nc.tensor.matmul(pblk[:], lhsT=ident16[:], rhs=svp_t[:],
                                     start=True, stop=(T == 0))
                    if T > 0:
                        w_t = pb.tile([128, TMAX, WJW], BF16, name="w_t")
                        nc.sync.dma_start(
                            out=w_t[:, 0:T, :],
                            in_=woh_d[t0 * 128:(t0 + T) * 128, :]
                            .rearrange("(j p) f -> p j f", p=128))
                        xvg = pb.tile([128, TMAX, 6 * H], BF16,
                                      name="xvg")
                        for c0 in range(0, TL, GCH):
                            cs = min(GCH, TL - c0)
                            nc.gpsimd.dma_gather(
                                out_ap=xvg[:, c0:c0 + cs, :],
                                in_ap=xvtab_d[:],
                                idxs_ap=idxw_t[:, 8 * (t0 + c0):
                                               8 * (t0 + c0 + cs)],
                                num_idxs=cs * 128,
                                num_idxs_reg=cs * 128,
                                elem_size=6 * H, queue_num=next_q())
                        for c0 in range(0, THI, GCH):
                            cs = min(GCH, THI - c0)
                            a0 = TL + c0
                            nc.gpsimd.dma_gather(
                                out_ap=xvg[:, a0:a0 + cs, :],
                                in_ap=xvtab_d[NSPLIT:NPAD, :],
                                idxs_ap=idxw_t[:, 8 * (t0 + a0):
                                               8 * (t0 + a0 + cs)],
                                num_idxs=cs * 128,
                                num_idxs_reg=cs * 128,
                                elem_size=6 * H, queue_num=next_q())
                        # per-edge message math (all bf16)
                        qm = pb.tile([128, TMAX, 896], BF16, name="qm")
                        t1d = pb.tile([128, TMAX, 3 * H], BF16, name="t1d")
                        # [ds|t1|t2] = [A|B|C] * [x0|x1|x2] in one op
                        nc.vector.tensor_tensor(
                            out=qm[:, 0:T, 384:768], in0=w_t[:, 0:T, 0:384],
                            in1=xvg[:, 0:T, 0:384], op=OP.mult)
                        # dv2 * vj  (t2 broadcast over the 3 dirs)
                        t2b = qm[:, 0:T, 640:768].unsqueeze(2).broadcast_to(
                            [128, T, 3, 128])
                        nc.vector.tensor_tensor(
                            out=qm[:, 0:T, 0:384].rearrange(
                                "p j (d f) -> p j d f", d=3),
                            in0=t2b, in1=xvg[:, 0:T, 3 * H:6 * H]
                            .rearrange("p j (d f) -> p j d f", d=3),
                            op=OP.mult)
                        # dv1 * dir_d
                        t1b = qm[:, 0:T, 512:640].unsqueeze(2).broadcast_to(
                            [128, T, 3, 128])
                        dirb = w_t[:, 0:T, 512:515].unsqueeze(3).broadcast_to(
                            [128, T, 3, 128])
                        nc.vector.tensor_tensor(
                            out=t1d[:, 0:T, :].rearrange(
                                "p j (d f) -> p j d f", d=3),
                            in0=t1b, in1=dirb, op=OP.mult)
                        nc.vector.tensor_tensor(
                            out=qm[:, 0:T, 0:384], in0=qm[:, 0:T, 0:384],
                            in1=t1d[:, 0:T, :], op=OP.add)
                        # scatter-accumulate into the block PSUM
                        for j in range(T):
                            nc.tensor.matmul(pblk[:],
                                             lhsT=w_t[:, j, 384:512],
                                             rhs=qm[:, j, 0:512],
                                             start=False,
                                             stop=(j == T - 1))
                    # ---- Phase C for block b ----
                    vsum = pcp.tile([128, 3 * H], BF16, name="vsum")
                    nc.scalar.activation(out=vsum[:], in_=pblk[:, 0:384],
                                         func=AF.Copy)
                    sv_raw = pcp.tile([128, H], BF16, name="sv_raw")
                    nc.scalar.activation(out=sv_raw[:], in_=pblk[:, 384:512],
                                         func=AF.Copy)
                    vws_all = pcp.tile([128, 3, 2 * H], BF16, name="vws_all")
                    for d in range(3):
                        trc = psc.tile([128, 128], BF16, name="trc",
                                       bufs=2)
                        nc.tensor.transpose(
                            trc[:],
                            in_=vsum[:, d * 128:(d + 1) * 128],
                            identity=ident16[:])
                        vT = pcp.tile([128, 128], BF16, name="vT", bufs=6)
                        nc.scalar.activation(out=vT[:], in_=trc[:],
                                             func=AF.Copy)
                        vw = psc.tile([128, 2 * H], F32, name="vw", bufs=2)
                        nc.tensor.matmul(vw[:], lhsT=vT[:],
                                         rhs=wvm_t[:],
                                         start=True, stop=True)
                        nc.scalar.activation(out=vws_all[:, d, :], in_=vw[:],
                                             func=AF.Copy)
                    prod2 = pcp.tile([128, 3, 128], F32, name="prod2")
                    nc.vector.tensor_tensor(out=prod2[:],
                                            in0=vws_all[:, :, 128:256],
                                            in1=vws_all[:, :, 128:256],
                                            op=OP.mult)
                    acc = pcp.tile([128, 128], F32, name="acc")
                    nc.vector.tensor_reduce(
                        out=acc[:], in_=prod2[:].rearrange("p d f -> p f d"),
                        axis=mybir.AxisListType.X, op=OP.add)
                    vnorm = pcp.tile([128, 128], BF16, name="vnorm")
                    nc.scalar.activation(out=vnorm[:], in_=acc[:],
                                         func=AF.Sqrt, bias=eps_t[:])
                    hps = psc.tile([128, 128], F32, name="hps")
                    for k, src_t in enumerate((sv_raw, vnorm)):
                        trc = psc.tile([128, 128], BF16, name="trc", bufs=2)
                        nc.tensor.transpose(trc[:],
                                            in_=src_t[:],
                                            identity=ident16[:])
                        tsT = pcp.tile([128, 128], BF16, name="tsT", bufs=4)
                        nc.scalar.activation(out=tsT[:], in_=trc[:],
                                             func=AF.Copy)
                        lhs = wm1a_t if k == 0 else wm1b_t
                        nc.tensor.matmul(hps[:], lhsT=lhs[:],
                                         rhs=tsT[:],
                                         start=(k == 0), stop=(k == 1))
                    hsb = pcp.tile([128, 128], BF16, name="hsb")
                    silu(pcp, hsb[:], hps[:], bm1_t[:], [128, 128], "sc")
                    ops_ = psc.tile([128, 3 * H], F32, name="ops")
                    nc.tensor.matmul(ops_[:], lhsT=ones1b[:],
                                     rhs=bm2_t[:],
                                     start=True, stop=False)
                    nc.tensor.matmul(ops_[:], lhsT=hsb[:],
                                     rhs=wm2_t[:],
                                     start=False, stop=True)
                    prodlv = pcp.tile([128, 3, 128], F32, name="prodlv")
                    nc.vector.tensor_tensor(out=prodlv[:],
                                            in0=vws_all[:, :, 0:128],
                                            in1=vws_all[:, :, 128:256],
                                            op=OP.mult)
                    svl = pcp.tile([128, 128], F32, name="svl")
                    nc.vector.tensor_reduce(
                        out=svl[:], in_=prodlv[:].rearrange("p d f -> p f d"),
                        axis=mybir.AxisListType.X, op=OP.add)
                    dsv = pcp.tile([128, 128], F32, name="dsv")
                    nc.vector.tensor_tensor(out=dsv[:], in0=ops_[:, 256:384],
                                            in1=svl[:], op=OP.mult)
                    accs = pcp.tile([128, 128], F32, name="accs")
                    nc.vector.tensor_tensor(out=accs[:], in0=ops_[:, 0:128],
                                            in1=dsv[:], op=OP.add)
                    outt = pcp.tile([128, 4 * H], BF16, name="outt")
                    t1 = pcp.tile([128, 128], F32, name="t1")
                    nc.scalar.activation(out=t1[:], in_=accs[:], func=AF.Copy,
                                         scale=EPSILON)
                    t2 = pcp.tile([128, 128], F32, name="t2")
                    nc.scalar.activation(out=t2[:], in_=pblk[:, 384:512],
                                         func=AF.Copy, scale=EPS2)
                    nc.vector.tensor_tensor(out=outt[:, 0:128], in0=t1[:],
                                            in1=t2[:], op=OP.add)
                    qall = pcp.tile([128, 3, 128], BF16, name="qall")
                    opd = ops_[:, 128:256].unsqueeze(1).broadcast_to(
                        [128, 3, 128])
                    nc.vector.tensor_tensor(out=qall[:],
                                            in0=vws_all[:, :, 0:128],
                                            in1=opd, op=OP.mult)
                    qs = pcp.tile([128, 3 * H], F32, name="qs")
                    nc.scalar.activation(
                        out=qs[:],
                        in_=qall[:].rearrange("p d f -> p (d f)"),
                        func=AF.Copy, scale=EPSILON)
                    r = pcp.tile([128, 3 * H], F32, name="r")
                    nc.scalar.activation(out=r[:], in_=pblk[:, 0:384],
                                         func=AF.Copy, scale=EPS2)
                    nc.vector.tensor_tensor(out=outt[:, 128:512],
                                            in0=qs[:], in1=r[:], op=OP.add)
                    nc.sync.dma_start(out=out_d[b * 128:(b + 1) * 128, :],
                                      in_=outt[:])


def _install_trace_hook():
    try:
        import antenv
        if "antenv.axon_hooks" not in sys.modules:
            mod = types.ModuleType("antenv.axon_hooks")
            mod._hook = None

            def set_axon_ntff_profile_hook(h):
                mod._hook = h

            def get_axon_ntff_profile_hook():
                return mod._hook

            mod.set_axon_ntff_profile_hook = set_axon_ntff_profile_hook
            mod.get_axon_ntff_profile_hook = get_axon_ntff_profile_hook
            sys.modules["antenv.axon_hooks"] = mod
            antenv.axon_hooks = mod
        from antenv.axon_hooks import (get_axon_ntff_profile_hook,
                                       set_axon_ntff_profile_hook)
        if get_axon_ntff_profile_hook() is None:
            from trn_agent_boot.trn_boot import _ntff_profile_via_ctypes
            set_axon_ntff_profile_hook(
                _ntff_profile_via_ctypes("/opt/axon/libaxon_pjrt.so"))
        return True
    except Exception:
        return False


def kernel(**inputs):
    from concourse import bacc
    from concourse.bass_utils import run_bass_kernel_spmd

    (shared, per_core, tiles_lo, tiles_hi, t_total,
     bi2_zero) = _preprocess(inputs)
    nc = bacc.Bacc("TRN2", target_bir_lowering=False, debug=False,
                   num_devices=NCORES, num_swdge_queues=4)
    _build(nc, tiles_lo, tiles_hi, t_total, bi2_zero)
    nc.compile()

    in_maps = [dict(shared, **per_core[c]) for c in range(NCORES)]
    trace = _install_trace_hook()
    try:
        res = run_bass_kernel_spmd(nc, in_maps, core_ids=list(range(NCORES)),
                                   trace=trace)
    except Exception:
        if not trace:
            raise
        res = run_bass_kernel_spmd(nc, in_maps, core_ids=list(range(NCORES)),
                                   trace=False)
    kernel.last_exec_time_ns = getattr(res, "exec_time_ns", None)
    outs = [np.asarray(res.results[c]["out"]) for c in range(NCORES)]
    full = np.concatenate(outs, axis=0)[:N_NODES]
    return np.ascontiguousarray(full.reshape(N_NODES, 4, H), dtype=np.float32)
